# revision 1
# baseline (speedup 1.0000x reference)
"""Bass/Tile TRN2 kernel for the MeanFieldGaussianLayer loss.

reference math:
    mean  = tensor[:, :, 0]                       (B, T)
    f_var = softplus(tensor[:, :, 1])
    y_var = f_var + softplus(noise) + 1e-6
    logp  = -0.5 * sum_T(LOG_2PI + log(y_var) + (y - mean)^2 / y_var)
    out   = mean_B(logp)

Strategy: pure data-parallel over B across 8 cores.  Each core gets
64 rows -> three fp32 planes [128, 8192] (t0 = mean, t1 = raw var, y).
Device computes, per partition, sum(log(y_var)) and sum(d^2 / y_var);
host combines the 8 x [128, 2] partials with the constant LOG_2PI term.

Per-element pipeline (a = exp(softplus(noise) + 1e-6)):
    ACT: u = Exp(t1); v = Ln(a*u + a)  [= softplus(t1) + c];
         Ln(v) with accum_out -> sum(log y_var); d2 = Square(d)
    DVE: r = reciprocal_approx_fast(v); d = y - t0;
         tensor_tensor_reduce(d2 * r) with accum_out -> sum(d^2 / y_var)
"""

import os
import sys

import numpy as np

if "/opt/trn_rl_repo" not in sys.path:
    sys.path.insert(0, "/opt/trn_rl_repo")

import concourse.bass as bass
import concourse.tile as tile
from concourse import bacc, mybir
from concourse import bass_utils

# ---------------------------------------------------------------------------
# Patch 1: force all ACT functions into the one table set that contains
# Exp+Ln+Square. bacc's insert_act_table_loads otherwise flip-flops between
# `exp_and_others` and `natural_log` (first-match), costing a ~1.3us
# ACT_TABLE_LOAD per switch (8 loads, ~10us, in the unpatched profile).
# ---------------------------------------------------------------------------
import concourse.bacc as _bacc_mod

_ACT_KEEP = "natural_log_exp_and_others"
_ACT_STRIP = {
    mybir.ActivationFunctionType.Exp,
    mybir.ActivationFunctionType.Ln,
    mybir.ActivationFunctionType.Square,
}
_orig_get_tables = _bacc_mod.get_activation_tables


def _patched_get_tables(arch):
    tabs = _orig_get_tables(arch)
    return {
        name: (set(fns) if name == _ACT_KEEP else set(fns) - _ACT_STRIP)
        for name, fns in tabs.items()
    }


_bacc_mod.get_activation_tables = _patched_get_tables

# ---------------------------------------------------------------------------
# Patch 1b: cheaper Tile kernel tail. The stock tail is
#   drain -> all_engine_barrier -> sem clears -> all_engine_barrier.
# Nothing executes after the tail in this single-TileContext kernel, so the
# trailing all-engine barrier only delays NEFF completion; drop it. Drain,
# first barrier, and the sem/DMA clears (re-execution safety) are kept.
# ---------------------------------------------------------------------------
import concourse.tile as _tile_mod
from concourse.vector_clock import ScopedClock as _ScopedClock


def _cheap_drain_and_barrier(self, tick_clock, wait_clock):
    drain_inst = self.nc.sync.drain()
    wait_clock.add_sem_waits(
        drain_inst.ins, _ScopedClock({None: tick_clock.global_clock})
    )
    self.nc.all_engine_barrier()
    popped = self.nc._tile_sem_poison_stack.pop()
    assert popped is self._sem_poison
    self.nc.clear_and_free_semaphores(list(self.sems.allocated().values()))


_tile_mod.TileContext._drain_and_barrier = _cheap_drain_and_barrier

# ---------------------------------------------------------------------------
# Patch 2: custom DVE op SQ_MUL_REDUCE_ANT:
#   out = (Src0^2) * Src1 ; accum_out = C0 + sum(out)
# One DVE pass for d^2 * r + reduction (replaces ACT Square + a second
# DVE pass). Registered at runtime into dve_ops' tables.
# ---------------------------------------------------------------------------
import concourse.dve_ops as _dve_ops
from concourse.dve_ops import DveOp, _ref_body_sum
from concourse.dve_spec import C0, Spec, Src0, Src1, _has_src1, lower, sq
from concourse.dve_uop import DveOpSpec
from operator import add as _op_add


def _register_sq_mul_reduce():
    name = "SQ_MUL_REDUCE_ANT"
    if name in _dve_ops._SUB_OPCODE_FOR_NAME:
        return next(op for op in _dve_ops.OPS if op.name == name)
    spec = Spec(
        body=sq(Src0) * Src1,
        accum=_op_add,
        accum_init=C0,
        reference=_ref_body_sum(
            lambda in0, in1, c0, c1, c2: np.square(in0.astype(np.float32)) * in1
        ),
    )
    row = max(_dve_ops._SUB_OPCODE_FOR_NAME.values()) + 1
    assert row < 0x20
    shas = {}
    for ver in ("v3", "v4"):
        try:
            uops = lower(spec, ver=ver)
            shas[ver] = DveOpSpec(
                name=name, opcode=row, uops=uops, rd1_en=_has_src1(spec)
            ).sha(ver)
        except Exception:
            pass
    op = DveOp(name, spec, subdim=False, uops_sha=shas)
    _dve_ops._SUB_OPCODE_FOR_NAME[name] = row
    _dve_ops.OPS.append(op)
    _dve_ops.CUSTOM_DVE_SPECS[name] = spec
    return op


SQ_MUL_REDUCE = _register_sq_mul_reduce()

B, T = 512, 16384
NCORES = 8
ROWS = B // NCORES          # 64 rows per core
P = 128                     # SBUF partitions
FPP = ROWS * T // P         # 8192 floats per partition per plane
# Tile free-dim schedule: big tiles for DMA efficiency, small final tiles
# so the last tile's serial compute chain (Exp->Ln->recip->sqmr) is short.
FDS = [2048, 2048, 2048, 1024, 512, 512]
assert sum(FDS) == 8192
NT = len(FDS)
FD_MAX = max(FDS)

LOG_2PI = float(np.log(2.0 * np.pi))
JITTER = 1e-6

_BUILD_CACHE: dict[float, object] = {}
LAST_RESULT = None  # BassKernelResults of the most recent run (for test harness)


def _build(a: float):
    """Build + compile the SPMD program. `a` = exp(softplus(noise) + jitter)."""
    f32 = mybir.dt.float32
    Act = mybir.ActivationFunctionType

    nc = bacc.Bacc("TRN2", target_bir_lowering=False, debug=False)

    t0 = nc.dram_tensor("t0", [P, FPP], f32, kind="ExternalInput").ap()
    t1 = nc.dram_tensor("t1", [P, FPP], f32, kind="ExternalInput").ap()
    y = nc.dram_tensor("y", [P, FPP], f32, kind="ExternalInput").ap()
    out = nc.dram_tensor("out", [P, 2], f32, kind="ExternalOutput").ap()

    with tile.TileContext(nc) as tc:
        with (
            tc.tile_pool(name="io", bufs=4) as io,
            tc.tile_pool(name="mid", bufs=2) as mid,
            tc.tile_pool(name="accs", bufs=1) as accs,
        ):
            acc_lg = accs.tile([P, NT], f32)   # per-tile sum(log v)
            acc_p = accs.tile([P, NT], f32)    # per-tile sum(d^2 / v)
            outt = accs.tile([P, 2], f32)
            abias = accs.tile([P, 1], f32)     # bias AP holding `a`
            nc.vector.memset(abias[:], a)
            # own zero-bias tile so no activation reads the init const APs
            # (lets us strip the init const memsets + boot barrier below)
            zbias = accs.tile([P, 1], f32)
            nc.vector.memset(zbias[:], 0.0)

            offs = [0]
            for FD in FDS:
                offs.append(offs[-1] + FD)
            sls = [slice(offs[i], offs[i + 1]) for i in range(NT)]

            # t1 runs one tile ahead in the DMA FIFO: the ACT chain (and the
            # reciprocal) for tile i finishes before y/t0 of tile i land, so
            # after the LAST y/t0 transfer only sub+sqmr remain.
            t1_tiles = {}
            t1_tiles[0] = io.tile([P, FDS[0]], f32, tag="t1", name="tt1_0")
            nc.sync.dma_start(t1_tiles[0][:], t1[:, sls[0]])

            off = 0
            for i in range(NT):
                FD = FDS[i]
                sl = sls[i]
                if i + 1 < NT:
                    t1_tiles[i + 1] = io.tile(
                        [P, FDS[i + 1]], f32, tag="t1", name=f"tt1_{i + 1}"
                    )
                    nc.sync.dma_start(t1_tiles[i + 1][:], t1[:, sls[i + 1]])
                tt1 = t1_tiles.pop(i)
                ty = io.tile([P, FD], f32, tag="y")
                nc.sync.dma_start(ty[:], y[:, sl])
                tt0 = io.tile([P, FD], f32, tag="t0")
                nc.sync.dma_start(tt0[:], t0[:, sl])

                # ACT chain: u = e^t1 ; v = ln(a*u + a) = softplus(t1) + c
                u = mid.tile([P, FD], f32, tag="u")
                nc.scalar.activation(u[:], tt1[:], Act.Exp, bias=zbias[:, 0:1])
                v = mid.tile([P, FD], f32, tag="v")
                nc.scalar.activation(v[:], u[:], Act.Ln, bias=abias[:, 0:1], scale=a)
                # sum(log v) via the ACT accumulator; elementwise output is
                # dead, overwrite u (its reader, the previous Ln, is done)
                nc.scalar.activation(
                    u[:],
                    v[:],
                    Act.Ln,
                    bias=zbias[:, 0:1],
                    accum_out=acc_lg[:, i : i + 1],
                )

                # d = y - t0. Big early tiles go to GpSimd (absorbs DVE's
                # backlog at 2.2x cost on an otherwise idle engine); small
                # late tiles stay on the faster DVE so the post-DMA tail
                # chain is short.
                d = mid.tile([P, FD], f32, tag="d")
                sub_eng = nc.gpsimd if FD > 1024 else nc.vector
                sub_eng.tensor_sub(d[:], ty[:], tt0[:])
                # DVE: r ~= 1/v  (~51 ULP; v >= ~0.69 so well-conditioned)
                r = mid.tile([P, FD], f32, tag="r")
                nc.vector.reciprocal_approx_fast(r[:], v[:])
                # fused d^2 * r + free-dim reduce in one custom DVE op
                scr2 = mid.tile([P, FD], f32, tag="scr2")
                nc.vector._custom_dve(
                    SQ_MUL_REDUCE,
                    out=scr2[:],
                    in0=d[:],
                    in1=r[:],
                    s0=0.0,
                    s1=0.0,
                    accum_out=acc_p[:, i : i + 1],
                )

            nc.vector.reduce_sum(outt[:, 0:1], acc_lg[:], axis=mybir.AxisListType.X)
            nc.vector.reduce_sum(outt[:, 1:2], acc_p[:], axis=mybir.AxisListType.X)
            nc.sync.dma_start(out[:], outt[:])

    nc.compile()
    return nc


def kernel(tensor, y_target, noise_unconstrained):
    global LAST_RESULT
    # scalar preprocessing on host (0-d input)
    noise = np.float64(np.asarray(noise_unconstrained))
    c = np.log1p(np.exp(-abs(noise))) + max(noise, 0.0) + JITTER  # softplus + jitter
    a = float(np.exp(c))

    key = a
    nc = _BUILD_CACHE.get(key)
    if nc is None:
        nc = _build(a)
        _BUILD_CACHE[key] = nc

    tensor = np.asarray(tensor, dtype=np.float32)
    y_target = np.asarray(y_target, dtype=np.float32)

    in_maps = []
    for k in range(NCORES):
        sh = tensor[k * ROWS : (k + 1) * ROWS]          # (64, 16384, 2)
        in_maps.append(
            {
                "t0": np.ascontiguousarray(sh[:, :, 0]).reshape(P, FPP),
                "t1": np.ascontiguousarray(sh[:, :, 1]).reshape(P, FPP),
                "y": np.ascontiguousarray(
                    y_target[k * ROWS : (k + 1) * ROWS, :, 0]
                ).reshape(P, FPP),
            }
        )

    trace = os.environ.get("BASS_KERNEL_PROFILE", "0") == "1"
    res = bass_utils.run_bass_kernel_spmd(
        nc, in_maps, list(range(NCORES)), trace=trace
    )
    LAST_RESULT = res

    total = np.float64(0.0)
    for k in range(NCORES):
        o = np.asarray(res.results[k]["out"], dtype=np.float64)
        total += o.sum()
    total += np.float64(B) * np.float64(T) * np.float64(LOG_2PI)
    return np.array(-0.5 * total / B, dtype=np.float32)



# revision 2
# speedup vs baseline: 1.2089x; 1.2089x over previous
"""Bass/Tile TRN2 kernel for the MeanFieldGaussianLayer loss.

reference math:
    mean  = tensor[:, :, 0]                       (B, T)
    f_var = softplus(tensor[:, :, 1])
    y_var = f_var + softplus(noise) + 1e-6
    logp  = -0.5 * sum_T(LOG_2PI + log(y_var) + (y - mean)^2 / y_var)
    out   = mean_B(logp)

Strategy: pure data-parallel over B across 8 cores.  Each core gets 64
rows -> planes of [128, 8192] fp32 (t1 = raw var; yt0 = per-chunk
interleaved [y_c | t0_c] so one DMA brings both sub operands).

Per-chunk pipeline (a = exp(softplus(noise) + 1e-6)):
    ACT: u = Exp(t1); v = Ln(a*u + a)  [= softplus(t1) + c];
         Ln(v) with accum_out -> sum(log y_var)
    DVE: d = y - t0; fused SQDIV_RED: (d^2 * ~1/v) with accum_out
         -> sum(d^2 / y_var)   (1-NR bit-trick reciprocal, ~0.2% max err)
Final: reduce chunk partials, PE ones-matmul folds 128 partitions ->
[1, 2], single-descriptor DMA out; host combines the 8 x [1, 2].

All input tiles are allocated fresh (no SBUF reuse -> DMA never waits);
every input descriptor is generated up front so the 16 DMA queues
stream back-to-back at ~HBM roofline.
"""

import os
import sys

import numpy as np

if "/opt/trn_rl_repo" not in sys.path:
    sys.path.insert(0, "/opt/trn_rl_repo")

import concourse.bass as bass
import concourse.tile as tile
from concourse import bacc, mybir
from concourse import bass_utils

# ---------------------------------------------------------------------------
# Patch 1: force all ACT functions into the one table set that contains
# Exp+Ln. bacc's insert_act_table_loads otherwise flip-flops between
# `exp_and_others` and `natural_log` (first-match), costing a ~1.3us
# ACT_TABLE_LOAD per switch.
# ---------------------------------------------------------------------------
import concourse.bacc as _bacc_mod

_ACT_KEEP = "natural_log_exp_and_others"
_ACT_STRIP = {
    mybir.ActivationFunctionType.Exp,
    mybir.ActivationFunctionType.Ln,
    mybir.ActivationFunctionType.Square,
}
_orig_get_tables = _bacc_mod.get_activation_tables


def _patched_get_tables(arch):
    tabs = _orig_get_tables(arch)
    return {
        name: (set(fns) if name == _ACT_KEEP else set(fns) - _ACT_STRIP)
        for name, fns in tabs.items()
    }


_bacc_mod.get_activation_tables = _patched_get_tables

# ---------------------------------------------------------------------------
# Patch 1b: cheaper Tile kernel tail. The stock tail is
#   drain -> all_engine_barrier -> sem clears -> all_engine_barrier.
# Nothing executes after the tail in this single-TileContext kernel, so the
# trailing all-engine barrier only delays NEFF completion; drop it. Drain,
# first barrier, and the sem/DMA clears (re-execution safety) are kept.
# ---------------------------------------------------------------------------
import concourse.tile as _tile_mod
from concourse.vector_clock import ScopedClock as _ScopedClock


def _cheap_drain_and_barrier(self, tick_clock, wait_clock):
    drain_inst = self.nc.sync.drain()
    wait_clock.add_sem_waits(
        drain_inst.ins, _ScopedClock({None: tick_clock.global_clock})
    )
    self.nc.all_engine_barrier()
    popped = self.nc._tile_sem_poison_stack.pop()
    assert popped is self._sem_poison
    self.nc.clear_and_free_semaphores(list(self.sems.allocated().values()))


_tile_mod.TileContext._drain_and_barrier = _cheap_drain_and_barrier

# ---------------------------------------------------------------------------
# Patch 2: custom DVE op SQDIV_RED_ANT:
#   out = Src0^2 * recip1(Src1) ; accum_out = C0 + sum(out)
# recip1 = one-NR bit-trick reciprocal (seed y0 = C0'*~bits(v), one Newton
# step y0*(C1 - v*y0); constants are the centered pair from
# RECIP_APPROX_FAST_CONSTS, max rel err ~0.18%).  One DVE pass replaces
# reciprocal_approx_fast + SQ_MUL_REDUCE of the previous design.
# ---------------------------------------------------------------------------
import concourse.dve_ops as _dve_ops
from concourse.dve_ops import DveOp, _ref_body_sum, RECIP_APPROX_FAST_CONSTS
from concourse.dve_spec import (
    C0,
    C1,
    Spec,
    Src0,
    Src1,
    _has_src1,
    lower,
    sq,
    AluOp,
)
from concourse.dve_spec import Bin as _Bin
from concourse.dve_uop import DveOpSpec
from operator import add as _op_add

RC0 = float(RECIP_APPROX_FAST_CONSTS["s0"])
RC1 = float(RECIP_APPROX_FAST_CONSTS["s1"])


def _register_custom(name, spec):
    if name in _dve_ops._SUB_OPCODE_FOR_NAME:
        return next(op for op in _dve_ops.OPS if op.name == name)
    row = max(_dve_ops._SUB_OPCODE_FOR_NAME.values()) + 1
    assert row < 0x20
    shas = {}
    for ver in ("v3", "v4"):
        try:
            uops = lower(spec, ver=ver)
            shas[ver] = DveOpSpec(
                name=name, opcode=row, uops=uops, rd1_en=_has_src1(spec)
            ).sha(ver)
        except Exception:
            pass
    assert shas, f"lower() failed for {name} on all vers"
    op = DveOp(name, spec, subdim=False, uops_sha=shas)
    _dve_ops._SUB_OPCODE_FOR_NAME[name] = row
    _dve_ops.OPS.append(op)
    _dve_ops.CUSTOM_DVE_SPECS[name] = spec
    return op


def _make_sqdiv_red():
    _not_v = _Bin(AluOp.BITWISE_NOT, Src1, Src1)
    _y0 = _not_v * C0
    _y1 = _y0 * (C1 - Src1 * _y0)
    body = sq(Src0) * _y1

    def _ref(in0, in1, c0, c1, c2):
        not_v = (~in1.astype(np.float32).view(np.int32)).view(np.float32)
        y0 = not_v * c0
        y1 = y0 * (c1 - in1 * y0)
        return np.square(in0.astype(np.float32)) * y1

    spec = Spec(
        body=body,
        accum=_op_add,
        accum_init=None,
        reference=_ref_body_sum(_ref),
    )
    return _register_custom("SQDIV_RED_ANT", spec)


SQDIV_RED = _make_sqdiv_red()

B, T = 512, 16384
NCORES = 8
ROWS = B // NCORES          # 64 rows per core
P = 128                     # SBUF partitions
FPP = ROWS * T // P         # 8192 floats per partition per plane
# Chunk schedule: 1024-wide steady state (good DMA lines, smooth engine
# pipelining), small final chunks so the post-last-DMA serial chain
# (sub -> sqdiv -> reduce -> out) is short.
FDS = [1024, 1024, 1024, 1024, 1024, 1024, 1024, 512, 256, 256]
assert sum(FDS) == FPP
NT = len(FDS)

LOG_2PI = float(np.log(2.0 * np.pi))
JITTER = 1e-6

_BUILD_CACHE: dict[float, object] = {}
LAST_RESULT = None  # BassKernelResults of the most recent run (for test harness)


def _strip_boot_preamble(nc):
    """Drop the const-AP memsets + boot all-engine barrier from the `main`
    bb.  Nothing in this kernel reads the const APs (all activation biases
    are explicit SBUF tiles), so the only effect of the preamble is ~1us of
    serial boot latency before the first DMA descriptor."""
    main_bb = nc.m.functions[0].blocks[0]
    assert main_bb.name == "main"
    drop = (mybir.InstMemset, mybir.InstDrain, mybir.InstEventSemaphore)
    kept = [i for i in main_bb.instructions if not isinstance(i, drop)]
    main_bb.instructions[:] = kept


def _build(a: float):
    """Build + compile the SPMD program. `a` = exp(softplus(noise) + jitter)."""
    f32 = mybir.dt.float32
    Act = mybir.ActivationFunctionType

    nc = bacc.Bacc("TRN2", target_bir_lowering=False, debug=False)
    _strip_boot_preamble(nc)

    t1 = nc.dram_tensor("t1", [P, FPP], f32, kind="ExternalInput").ap()
    yt0 = nc.dram_tensor("yt0", [P, 2 * FPP], f32, kind="ExternalInput").ap()
    out = nc.dram_tensor("out", [1, 2], f32, kind="ExternalOutput").ap()

    offs = [0]
    for FD in FDS:
        offs.append(offs[-1] + FD)

    with tile.TileContext(nc) as tc:
        with (
            tc.tile_pool(name="io", bufs=1) as io,
            tc.tile_pool(name="mid", bufs=2) as mid,
            tc.tile_pool(name="accs", bufs=1) as accs,
            tc.tile_pool(name="psum", bufs=1, space=bass.MemorySpace.PSUM) as psum,
        ):
            acc = accs.tile([P, 2 * NT], f32)   # [lg sums | d2/v sums]
            outt = accs.tile([P, 2], f32)
            outs = accs.tile([1, 2], f32)
            pacc = psum.tile([1, 2], f32)
            abias = accs.tile([P, 1], f32)      # bias AP holding `a`
            nc.vector.memset(abias[:], a)
            # own zero-bias tile so no activation reads the init const APs
            zbias = accs.tile([P, 1], f32)
            nc.vector.memset(zbias[:], 0.0)
            ones = accs.tile([P, 1], f32)
            nc.vector.memset(ones[:], 1.0)

            # All input tiles are fresh allocations -> dma_starts carry no
            # waits; issue every descriptor up front (t1 two chunks ahead
            # of its consumer, yt0 in chunk order behind it).
            t1_t, yt0_t = {}, {}

            def dma_t1(i):
                t1_t[i] = io.tile([P, FDS[i]], f32, tag=f"t1_{i}", name=f"tt1_{i}")
                nc.sync.dma_start(t1_t[i][:], t1[:, offs[i] : offs[i + 1]])

            def dma_yt0(i):
                yt0_t[i] = io.tile(
                    [P, 2 * FDS[i]], f32, tag=f"yt0_{i}", name=f"tyt0_{i}"
                )
                nc.sync.dma_start(
                    yt0_t[i][:], yt0[:, 2 * offs[i] : 2 * offs[i + 1]]
                )

            dma_t1(0)
            dma_t1(1)
            for i in range(NT):
                dma_yt0(i)
                if i + 2 < NT:
                    dma_t1(i + 2)

            for i in range(NT):
                FD = FDS[i]
                tt1 = t1_t.pop(i)
                tyt = yt0_t.pop(i)

                # ACT chain: u = e^t1 ; v = ln(a*u + a) = softplus(t1) + c
                u = mid.tile([P, FD], f32, tag="u")
                nc.scalar.activation(u[:], tt1[:], Act.Exp, bias=zbias[:, 0:1])
                v = mid.tile([P, FD], f32, tag="v", bufs=3)
                nc.scalar.activation(v[:], u[:], Act.Ln, bias=abias[:, 0:1], scale=a)
                # sum(log v) via the ACT accumulator; elementwise output is
                # dead, overwrite u (its reader, the previous Ln, is done)
                nc.scalar.activation(
                    u[:],
                    v[:],
                    Act.Ln,
                    bias=zbias[:, 0:1],
                    accum_out=acc[:, i : i + 1],
                )

                # DVE: d = y - t0 (both halves of the interleaved tile)
                d = mid.tile([P, FD], f32, tag="d")
                nc.vector.tensor_sub(d[:], tyt[:, 0:FD], tyt[:, FD : 2 * FD])
                # fused d^2 * ~(1/v) + free-dim reduce in one custom DVE op
                scr = mid.tile([P, FD], f32, tag="scr")
                nc.vector._custom_dve(
                    SQDIV_RED,
                    out=scr[:],
                    in0=d[:],
                    in1=v[:],
                    s0=RC0,
                    s1=RC1,
                    accum_out=acc[:, NT + i : NT + i + 1],
                )

            nc.vector.reduce_sum(outt[:, 0:1], acc[:, 0:NT], axis=mybir.AxisListType.X)
            nc.vector.reduce_sum(
                outt[:, 1:2], acc[:, NT : 2 * NT], axis=mybir.AxisListType.X
            )
            # fold 128 partitions -> 1 with a ones-matmul so the output DMA
            # is a single descriptor instead of a 128-descriptor storm
            nc.tensor.matmul(pacc[:, :], ones[:, 0:1], outt[:, 0:2])
            nc.vector.tensor_copy(outs[:, :], pacc[:, :])
            nc.sync.dma_start(out[:], outs[:])

    nc.compile()
    return nc


def kernel(tensor, y_target, noise_unconstrained):
    global LAST_RESULT
    # scalar preprocessing on host (0-d input)
    noise = np.float64(np.asarray(noise_unconstrained))
    c = np.log1p(np.exp(-abs(noise))) + max(noise, 0.0) + JITTER  # softplus + jitter
    a = float(np.exp(c))

    key = a
    nc = _BUILD_CACHE.get(key)
    if nc is None:
        nc = _build(a)
        _BUILD_CACHE[key] = nc

    tensor = np.asarray(tensor, dtype=np.float32)
    y_target = np.asarray(y_target, dtype=np.float32)

    offs = [0]
    for FD in FDS:
        offs.append(offs[-1] + FD)

    in_maps = []
    for k in range(NCORES):
        sh = tensor[k * ROWS : (k + 1) * ROWS]          # (64, 16384, 2)
        t1p = np.ascontiguousarray(sh[:, :, 1]).reshape(P, FPP)
        yp = np.ascontiguousarray(
            y_target[k * ROWS : (k + 1) * ROWS, :, 0]
        ).reshape(P, FPP)
        t0p = np.ascontiguousarray(sh[:, :, 0]).reshape(P, FPP)
        yt0p = np.empty((P, 2 * FPP), dtype=np.float32)
        for i in range(NT):
            lo, hi = offs[i], offs[i + 1]
            yt0p[:, 2 * lo : lo + hi] = yp[:, lo:hi]
            yt0p[:, lo + hi : 2 * hi] = t0p[:, lo:hi]
        in_maps.append({"t1": t1p, "yt0": yt0p})

    trace = os.environ.get("BASS_KERNEL_PROFILE", "0") == "1"
    res = bass_utils.run_bass_kernel_spmd(
        nc, in_maps, list(range(NCORES)), trace=trace
    )
    LAST_RESULT = res

    total = np.float64(0.0)
    for k in range(NCORES):
        o = np.asarray(res.results[k]["out"], dtype=np.float64)
        total += o.sum()
    total += np.float64(B) * np.float64(T) * np.float64(LOG_2PI)
    return np.array(-0.5 * total / B, dtype=np.float32)
